# revision 13
# baseline (speedup 1.0000x reference)
"""Causal dilated 1D conv (B=16, C=32, L=131072, KW=3, dil=4, left-pad 8)
as a Bass/Tile kernel on 8 Trainium2 NeuronCores.

Strategy
--------
Data-parallel: batch dim 16 -> 2 batches per core; weights replicated.

Phase-domain packing: with dilation 4, decompose t = 4j + s.  For each
phase s the conv is a *dense* causal kw=3 conv on the subsampled
sequence x_s[j] = x[:, 4j+s]:

    y_s[j] = sum_k W_k x_s[j-2+k]

Blocking j = 4m + r (r = 0..3) and packing partition p = 32 r + c gives

    Y[m] = A @ X[m] + B @ X[m-1]

with A, B two 128x128 host-built matrices (A holds the 9 intra-block
(r_out, r_in) tap blocks, B the 3 cross-block ones).  Two 512-col
matmuls per PSUM bank replace a naive scheme's three; PE density 37.5%.

The host pre-permutes x into exactly the SBUF tile layout (one
contiguous [128, w] fp16 blob per tile), so every DMA both ways is a
maximal contiguous stream of 8192-byte 512B-aligned runs -- the kernel
is DMA-bound at ~420 GB/s aggregate (16 SDMA engines x ~26 GB/s).
The one-column halo the B matmul needs at each tile seam is NOT
shipped twice: the seam PSUM column is patched with a 1-wide matmul
against the previous tile's last column (still resident in SBUF).

PSUM->SBUF fp32->fp16 casts alternate between the vector and scalar
(activation) engines, each draining a 4-bank [128, 2048] PSUM half in
one instruction; output DMA is issued from the scalar engine's queue.
First/last phase rows use tapered tile widths for faster ramp/drain.
"""

import numpy as np

import concourse.bass as bass
import concourse.mybir as mybir
from concourse.tile import TileContext
from concourse.bass_utils import run_bass_kernel_spmd

# Problem constants (hardcoded per harness contract).
B, C, L = 16, 32, 131072
KW, DIL, PAD = 3, 4, 8

N_CORES = 8
B_PER_CORE = B // N_CORES          # 2
R = 4                              # sub-times per block (partition groups)
S = DIL                            # phases
M = L // (R * S)                   # 8192 block-columns per phase row
ROWS = B_PER_CORE * S              # 8 phase rows per core
GTILE = 4096                       # block-columns per x tile
HALF = 2048                        # psum half width (4 fp32 banks)
MM_N = 512                         # one PSUM bank of fp32
RUN = 4096                         # max elems per DMA descriptor run (8 KiB)

DT = mybir.dt.float16
NPDT = np.float16


def _row_widths(row: int) -> list[int]:
    # taper: small first tiles (fast ramp) / small last (fast drain)
    if row == 0:
        return [1024, 3072, GTILE]
    if row == ROWS - 1:
        return [GTILE, 3072, 1024]
    return [GTILE, GTILE]


def _dma_dims(gw: int) -> list[list[int]]:
    """DRAM-side AP for a [128, gw] tile blob: split each partition row
    into 8 KiB descriptor runs (same linear element order) so the DGE
    spreads many medium runs over all 16 SDMA engines."""
    run = min(RUN, gw)
    return [[run, 128 * (gw // run)], [1, run]]


def _tiles():
    """(row, col0, width, elem_offset) per tile; x and out share layout."""
    off = 0
    out = []
    for row in range(ROWS):
        c0 = 0
        for w in _row_widths(row):
            out.append((row, c0, w, off))
            off += 128 * w
            c0 += w
    return out, off


TILES, TOT = _tiles()


def _split_sync_waits(nc: bass.Bass, max_waits: int = 1) -> None:
    """The walrus build in this container rejects >`max_waits` sync-waits on
    an instruction.  Hoist excess waits onto fresh NoOp instructions inserted
    just before the offender on the same engine -- program order on one engine
    serializes them, so semantics are unchanged."""
    ctr = 0
    for f in nc.m.functions:
        for bb in f.blocks:
            insts = bb.instructions
            new = []
            for inst in insts:
                si = getattr(inst, "sync_info", None)
                if si is not None and si.on_wait and len(si.on_wait) > max_waits:
                    waits = list(si.on_wait)
                    head, keep = waits[:-max_waits], waits[-max_waits:]
                    for w in head:
                        nop = mybir.InstNoOp(
                            name=f"splitw-{ctr}",
                            engine=inst.engine,
                            bass_nofuse=True,
                            sync_info=mybir.SyncInfo(on_wait=[w], on_update=[]),
                        )
                        ctr += 1
                        new.append(nop)
                    inst.sync_info = mybir.SyncInfo(
                        on_wait=keep, on_update=list(si.on_update or [])
                    )
                new.append(inst)
            insts[:] = new


def _build_nc() -> bass.Bass:
    nc = bass.Bass(target_bir_lowering=False, trn_type="TRN2")
    x = nc.dram_tensor("x", [TOT], DT, kind="ExternalInput")
    w = nc.dram_tensor("w", [128, 2, 128], DT, kind="ExternalInput")
    out = nc.dram_tensor("out", [TOT], DT, kind="ExternalOutput")

    with TileContext(nc) as tc:
        with (
            tc.tile_pool(name="wpool", bufs=1) as wpool,
            tc.tile_pool(name="xpool", bufs=4) as xpool,
            tc.tile_pool(name="opool", bufs=4) as opool,
            tc.tile_pool(name="psum", bufs=2, space="PSUM") as psum,
        ):
            wt = wpool.tile([128, 2, 128], DT)
            nc.sync.dma_start(out=wt[:], in_=w[:])

            cast_tgl = 0
            prev_xt = None
            prev_gw = 0
            for _row, c0, gw, off in TILES:
                xt = xpool.tile([128, GTILE], DT, name="xt")
                nc.sync.dma_start(
                    out=xt[:, :gw],
                    in_=bass.AP(x, off, _dma_dims(gw)),
                )

                ot = opool.tile([128, GTILE], DT, name="ot")
                pos = 0
                while pos < gw:
                    half = min(HALF, gw - pos)
                    pt = psum.tile([128, HALF], mybir.dt.float32, name="pt")
                    for j in range(half // MM_N):
                        a0 = pos + j * MM_N
                        po = pt[:, j * MM_N : (j + 1) * MM_N]
                        nc.tensor.matmul(
                            out=po,
                            lhsT=wt[:, 0, :],
                            rhs=xt[:, a0 : a0 + MM_N],
                            start=True,
                            stop=False,
                        )
                        if a0 > 0:
                            # B matmul: rhs is the same tile shifted one
                            # block-column left.
                            nc.tensor.matmul(
                                out=po,
                                lhsT=wt[:, 1, :],
                                rhs=xt[:, a0 - 1 : a0 - 1 + MM_N],
                                start=False,
                                stop=True,
                            )
                        else:
                            # Tile seam: column 0's B operand is the previous
                            # tile's last column (zero at a phase-row start).
                            nc.tensor.matmul(
                                out=pt[:, 1:MM_N],
                                lhsT=wt[:, 1, :],
                                rhs=xt[:, 0 : MM_N - 1],
                                start=False,
                                stop=(c0 == 0),
                                skip_group_check=True,
                            )
                            if c0 > 0:
                                nc.tensor.matmul(
                                    out=pt[:, 0:1],
                                    lhsT=wt[:, 1, :],
                                    rhs=prev_xt[:, prev_gw - 1 : prev_gw],
                                    start=False,
                                    stop=True,
                                    skip_group_check=True,
                                )
                    dst = ot[:, pos : pos + half]
                    if cast_tgl == 0:
                        nc.vector.tensor_copy(out=dst, in_=pt[:, :half])
                    else:
                        nc.scalar.copy(out=dst, in_=pt[:, :half])
                    cast_tgl ^= 1
                    pos += half

                nc.scalar.dma_start(
                    out=bass.AP(out, off, _dma_dims(gw)),
                    in_=ot[:, :gw],
                )
                prev_xt, prev_gw = xt, gw
    _split_sync_waits(nc)
    return nc


_NC_CACHE = None


def _get_nc() -> bass.Bass:
    global _NC_CACHE
    if _NC_CACHE is None:
        _NC_CACHE = _build_nc()
    return _NC_CACHE


def _build_weights(W: np.ndarray) -> np.ndarray:
    """lhsT pair [p_in, {A,B}, p_out] for the phase-block scheme."""
    Wk = W.reshape(C, C, KW)  # (co, ci, k)
    A = np.zeros((128, 128), np.float32)
    Bm = np.zeros((128, 128), np.float32)
    for r_o in range(R):
        for r_i in range(R):
            k = r_i + 2 - r_o
            if 0 <= k < KW:
                A[r_o * C : (r_o + 1) * C, r_i * C : (r_i + 1) * C] = Wk[:, :, k]
            k = r_i - 2 - r_o
            if 0 <= k < KW:
                Bm[r_o * C : (r_o + 1) * C, r_i * C : (r_i + 1) * C] = Wk[:, :, k]
    return np.ascontiguousarray(
        np.stack([A.T, Bm.T], axis=1).astype(NPDT)  # [p_in, 2, p_out]
    )


def kernel(x: np.ndarray, W: np.ndarray, _trace: bool = False):
    x = np.ascontiguousarray(x, dtype=np.float32)   # (16, 32, 131072)
    W = np.ascontiguousarray(W, dtype=np.float32)   # (32, 96)

    wab = _build_weights(W)

    # X_dev[core][row = b*S + s, p = 32 r + c, m] = x[b, c, 16 m + 4 r + s]
    x16 = x.astype(NPDT).reshape(N_CORES, B_PER_CORE, C, M, R, S)
    xdev = np.ascontiguousarray(
        x16.transpose(0, 1, 5, 4, 2, 3).reshape(N_CORES, ROWS, 128, M)
    )

    nc = _get_nc()
    in_maps = []
    for core in range(N_CORES):
        blob = np.empty(TOT, dtype=NPDT)
        for row, c0, gw, off in TILES:
            blob[off : off + 128 * gw].reshape(128, gw)[:] = xdev[
                core, row, :, c0 : c0 + gw
            ]
        in_maps.append({"x": blob, "w": wab})

    res = run_bass_kernel_spmd(
        nc, in_maps, core_ids=list(range(N_CORES)), trace=_trace
    )

    odev = np.empty((N_CORES, ROWS, 128, M), dtype=NPDT)
    for core in range(N_CORES):
        o = res.results[core]["out"]
        for row, c0, gw, off in TILES:
            odev[core, row, :, c0 : c0 + gw] = o[off : off + 128 * gw].reshape(
                128, gw
            )

    # out[b, c, 16 m + 4 r + s] = odev[row = b*S+s, 32 r + c, m]
    out = np.ascontiguousarray(
        odev.reshape(N_CORES, B_PER_CORE, S, R, C, M)
        .transpose(0, 1, 4, 5, 3, 2)
        .reshape(B, C, L)
        .astype(np.float32)
    )
    if _trace:
        return out, res
    return out


# revision 14
# speedup vs baseline: 1.1019x; 1.1019x over previous
"""Causal dilated 1D conv (B=16, C=32, L=131072, KW=3, dil=4, left-pad 8)
as a Bass/Tile kernel on 8 Trainium2 NeuronCores.

Strategy
--------
Data-parallel: batch dim 16 -> 2 batches per core; weights replicated.

Phase-domain packing: with dilation 4, decompose t = 4j + s.  For each
phase s the conv is a *dense* causal kw=3 conv on the subsampled
sequence x_s[j] = x[:, 4j+s]:

    y_s[j] = sum_k W_k x_s[j-2+k]

Blocking j = 4m + r (r = 0..3) and packing partition p = 32 r + c gives

    Y[m] = A @ X[m] + B @ X[m-1]

with A, B two 128x128 host-built matrices (A holds the 9 intra-block
(r_out, r_in) tap blocks, B the 3 cross-block ones).  Two 512-col
matmuls per PSUM bank replace a naive scheme's three; PE density 37.5%.

The host pre-permutes x into exactly the SBUF tile layout (one
contiguous [128, w] fp16 blob per tile), so every DMA both ways is a
maximal contiguous stream of 8192-byte 512B-aligned runs -- the kernel
is DMA-bound at ~420 GB/s aggregate (16 SDMA engines x ~26 GB/s).
The one-column halo the B matmul needs at each tile seam is NOT
shipped twice: the seam PSUM column is patched with a 1-wide matmul
against the previous tile's last column (still resident in SBUF).

PSUM->SBUF fp32->fp16 casts alternate between the vector and scalar
(activation) engines, each draining a 4-bank [128, 2048] PSUM half in
one instruction; output DMA is issued from the scalar engine's queue.
First/last phase rows use tapered tile widths for faster ramp/drain.
"""

import numpy as np

import concourse.bass as bass
import concourse.mybir as mybir
from concourse.tile import TileContext
from concourse.bass_utils import run_bass_kernel_spmd

# Problem constants (hardcoded per harness contract).
B, C, L = 16, 32, 131072
KW, DIL, PAD = 3, 4, 8

N_CORES = 8
B_PER_CORE = B // N_CORES          # 2
R = 4                              # sub-times per block (partition groups)
S = DIL                            # phases
M = L // (R * S)                   # 8192 block-columns per phase row
ROWS = B_PER_CORE * S              # 8 phase rows per core
GTILE = 4096                       # block-columns per x tile
HALF = 2048                        # psum half width (4 fp32 banks)
MM_N = 512                         # one PSUM bank of fp32
RUN = 4096                         # max elems per DMA descriptor run (8 KiB)

DT = mybir.dt.float16
NPDT = np.float16


def _row_widths(row: int) -> list[int]:
    # taper: small first tiles (fast ramp) / small last (fast drain)
    if row == 0:
        return [1024, 3072, GTILE]
    if row == ROWS - 1:
        return [GTILE, 3072, 1024]
    return [GTILE, GTILE]


def _dma_dims(gw: int) -> list[list[int]]:
    """DRAM-side AP for a [128, gw] tile blob: split each partition row
    into 8 KiB descriptor runs (same linear element order) so the DGE
    spreads many medium runs over all 16 SDMA engines."""
    run = min(RUN, gw)
    return [[run, 128 * (gw // run)], [1, run]]


def _tiles():
    """(row, col0, width, elem_offset) per tile; x and out share layout."""
    off = 0
    out = []
    for row in range(ROWS):
        c0 = 0
        for w in _row_widths(row):
            out.append((row, c0, w, off))
            off += 128 * w
            c0 += w
    return out, off


TILES, TOT = _tiles()


def _split_sync_waits(nc: bass.Bass, max_waits: int = 1) -> None:
    """The walrus build in this container rejects >`max_waits` sync-waits on
    an instruction.  Hoist excess waits onto fresh NoOp instructions inserted
    just before the offender on the same engine -- program order on one engine
    serializes them, so semantics are unchanged."""
    ctr = 0
    for f in nc.m.functions:
        for bb in f.blocks:
            insts = bb.instructions
            new = []
            for inst in insts:
                si = getattr(inst, "sync_info", None)
                if si is not None and si.on_wait and len(si.on_wait) > max_waits:
                    waits = list(si.on_wait)
                    head, keep = waits[:-max_waits], waits[-max_waits:]
                    for w in head:
                        nop = mybir.InstNoOp(
                            name=f"splitw-{ctr}",
                            engine=inst.engine,
                            bass_nofuse=True,
                            sync_info=mybir.SyncInfo(on_wait=[w], on_update=[]),
                        )
                        ctr += 1
                        new.append(nop)
                    inst.sync_info = mybir.SyncInfo(
                        on_wait=keep, on_update=list(si.on_update or [])
                    )
                new.append(inst)
            insts[:] = new


def _build_nc() -> bass.Bass:
    nc = bass.Bass(target_bir_lowering=False, trn_type="TRN2")
    x = nc.dram_tensor("x", [TOT], DT, kind="ExternalInput")
    w = nc.dram_tensor("w", [128, 2, 128], DT, kind="ExternalInput")
    out = nc.dram_tensor("out", [TOT], DT, kind="ExternalOutput")

    with TileContext(nc) as tc:
        with (
            tc.tile_pool(name="wpool", bufs=1) as wpool,
            tc.tile_pool(name="xpool", bufs=6) as xpool,
            tc.tile_pool(name="opool", bufs=6) as opool,
            tc.tile_pool(name="psum", bufs=2, space="PSUM") as psum,
        ):
            wt = wpool.tile([128, 2, 128], DT)
            nc.sync.dma_start(out=wt[:], in_=w[:])

            cast_tgl = 0
            prev_xt = None
            prev_gw = 0
            for _row, c0, gw, off in TILES:
                xt = xpool.tile([128, GTILE], DT, name="xt")
                nc.sync.dma_start(
                    out=xt[:, :gw],
                    in_=bass.AP(x, off, _dma_dims(gw)),
                )

                ot = opool.tile([128, GTILE], DT, name="ot")
                pos = 0
                while pos < gw:
                    half = min(HALF, gw - pos)
                    pt = psum.tile([128, HALF], mybir.dt.float32, name="pt")
                    for j in range(half // MM_N):
                        a0 = pos + j * MM_N
                        po = pt[:, j * MM_N : (j + 1) * MM_N]
                        nc.tensor.matmul(
                            out=po,
                            lhsT=wt[:, 0, :],
                            rhs=xt[:, a0 : a0 + MM_N],
                            start=True,
                            stop=False,
                        )
                        if a0 > 0:
                            # B matmul: rhs is the same tile shifted one
                            # block-column left.
                            nc.tensor.matmul(
                                out=po,
                                lhsT=wt[:, 1, :],
                                rhs=xt[:, a0 - 1 : a0 - 1 + MM_N],
                                start=False,
                                stop=True,
                            )
                        else:
                            # Tile seam: column 0's B operand is the previous
                            # tile's last column (zero at a phase-row start).
                            nc.tensor.matmul(
                                out=pt[:, 1:MM_N],
                                lhsT=wt[:, 1, :],
                                rhs=xt[:, 0 : MM_N - 1],
                                start=False,
                                stop=(c0 == 0),
                                skip_group_check=True,
                            )
                            if c0 > 0:
                                nc.tensor.matmul(
                                    out=pt[:, 0:1],
                                    lhsT=wt[:, 1, :],
                                    rhs=prev_xt[:, prev_gw - 1 : prev_gw],
                                    start=False,
                                    stop=True,
                                    skip_group_check=True,
                                )
                    dst = ot[:, pos : pos + half]
                    if cast_tgl == 0:
                        nc.vector.tensor_copy(out=dst, in_=pt[:, :half])
                    else:
                        nc.scalar.copy(out=dst, in_=pt[:, :half])
                    cast_tgl ^= 1
                    pos += half

                nc.scalar.dma_start(
                    out=bass.AP(out, off, _dma_dims(gw)),
                    in_=ot[:, :gw],
                )
                prev_xt, prev_gw = xt, gw
    _split_sync_waits(nc)
    return nc


_NC_CACHE = None


def _get_nc() -> bass.Bass:
    global _NC_CACHE
    if _NC_CACHE is None:
        _NC_CACHE = _build_nc()
    return _NC_CACHE


def _build_weights(W: np.ndarray) -> np.ndarray:
    """lhsT pair [p_in, {A,B}, p_out] for the phase-block scheme."""
    Wk = W.reshape(C, C, KW)  # (co, ci, k)
    A = np.zeros((128, 128), np.float32)
    Bm = np.zeros((128, 128), np.float32)
    for r_o in range(R):
        for r_i in range(R):
            k = r_i + 2 - r_o
            if 0 <= k < KW:
                A[r_o * C : (r_o + 1) * C, r_i * C : (r_i + 1) * C] = Wk[:, :, k]
            k = r_i - 2 - r_o
            if 0 <= k < KW:
                Bm[r_o * C : (r_o + 1) * C, r_i * C : (r_i + 1) * C] = Wk[:, :, k]
    return np.ascontiguousarray(
        np.stack([A.T, Bm.T], axis=1).astype(NPDT)  # [p_in, 2, p_out]
    )


def kernel(x: np.ndarray, W: np.ndarray, _trace: bool = False):
    x = np.ascontiguousarray(x, dtype=np.float32)   # (16, 32, 131072)
    W = np.ascontiguousarray(W, dtype=np.float32)   # (32, 96)

    wab = _build_weights(W)

    # X_dev[core][row = b*S + s, p = 32 r + c, m] = x[b, c, 16 m + 4 r + s]
    x16 = x.astype(NPDT).reshape(N_CORES, B_PER_CORE, C, M, R, S)
    xdev = np.ascontiguousarray(
        x16.transpose(0, 1, 5, 4, 2, 3).reshape(N_CORES, ROWS, 128, M)
    )

    nc = _get_nc()
    in_maps = []
    for core in range(N_CORES):
        blob = np.empty(TOT, dtype=NPDT)
        for row, c0, gw, off in TILES:
            blob[off : off + 128 * gw].reshape(128, gw)[:] = xdev[
                core, row, :, c0 : c0 + gw
            ]
        in_maps.append({"x": blob, "w": wab})

    res = run_bass_kernel_spmd(
        nc, in_maps, core_ids=list(range(N_CORES)), trace=_trace
    )

    odev = np.empty((N_CORES, ROWS, 128, M), dtype=NPDT)
    for core in range(N_CORES):
        o = res.results[core]["out"]
        for row, c0, gw, off in TILES:
            odev[core, row, :, c0 : c0 + gw] = o[off : off + 128 * gw].reshape(
                128, gw
            )

    # out[b, c, 16 m + 4 r + s] = odev[row = b*S+s, 32 r + c, m]
    out = np.ascontiguousarray(
        odev.reshape(N_CORES, B_PER_CORE, S, R, C, M)
        .transpose(0, 1, 4, 5, 3, 2)
        .reshape(B, C, L)
        .astype(np.float32)
    )
    if _trace:
        return out, res
    return out
